# revision 16
# baseline (speedup 1.0000x reference)
"""Trainium2 Bass kernel for nn_BackwardConv2D (conv_transpose backward pass).

Math (verified vs jax): for each (batch, n_out) slice, with z = w reshaped to
[H, W, C_OUT] (channels-last, flat index (y*W + x)*C_OUT + co):

    out[y, x, ci] = sum_{a,b in 0..2} z_pad[y+a-1, x+b-1, co] * kf[a, b, ci, co]
    kf = kernel[::-1, ::-1]            (spatial flip; zero 'SAME' padding)
    b_new[n]     = sum_{y,x,co} z[y,x,co] * bias[co] + b[n]

Sharding: data-parallel over batch B=8 -> one batch slice per NeuronCore
(each core handles both the _u and _l tensors of its batch).

Per-core implementation: the co=64 contraction is packed two x-columns at a
time into the PE array's K=128 partitions ("odd-pair" tiling: pair k holds
input columns x=2k+1, 2k+2, so the three K-spans feeding a 4-wide output
x-group are all pair-aligned).  Each PSUM fill covers outputs
[128 part = (x%4, ci), 512 free = (16 y, 32 n)] and accumulates 9 matmuls
(3 K-spans x 3 dy-taps; dy shifts are free-dim offsets).  Matmuls run as
float32r (1 cycle/row vs 4 for float32).  The bias reduction reuses the same
SBUF tiles with an M=1 matmul against a replicated bias column.  x-edge taps
are handled by zeroed lhsT variants, y-edge taps by trimming the free range.
All lhsT weight matrices are built host-side from the small kernel/bias
inputs and shipped as one [128, 1923] DRAM tensor.
"""

from contextlib import ExitStack

import numpy as np

import concourse.bass as bass
import concourse.mybir as mybir
import concourse.tile as tile
from concourse import bacc
from concourse.bass_utils import run_bass_kernel_spmd

H = W = 32
CI, CO = 32, 64
B, NOUT = 8, 32
NF_OUT = H * W * CO   # 65536
NF_IN = H * W * CI    # 32768
F32 = mybir.dt.float32
WK_COLS = 15 * 128 + 3  # 15 lhsT tiles + 3 bias columns


def _host_weights(kernel_np: np.ndarray, bias_np: np.ndarray) -> np.ndarray:
    """Build the [128, 1923] stationary-weight matrix from kernel/bias.

    lhsT tile for K-span s (s in 0..2), dy-tap a: [K=128, M=128] where
    K rows = (half, co) for input columns x = 4g-1+2s+half and
    M cols = (j, ci) for output column x = 4g+j.  The x-tap index is
    bt = 2s + half - j (valid 0..2).  Tiles 3/4 are edge variants of
    s=0/s=2 with the out-of-range input half zeroed (x=-1 / x=32).
    """
    kf = kernel_np[::-1, ::-1]  # [a, bt, ci, co]

    def span_lhs(s, zero_half=None):
        out = np.zeros((3, 128, 128), np.float32)
        for a in range(3):
            for half in range(2):
                if half == zero_half:
                    continue
                for j in range(4):
                    bt = 2 * s + half - j
                    if 0 <= bt <= 2:
                        out[a, half * 64:(half + 1) * 64, j * 32:(j + 1) * 32] = kf[a, bt].T
        return out

    tiles = [span_lhs(0), span_lhs(1), span_lhs(2),
             span_lhs(0, zero_half=0), span_lhs(2, zero_half=1)]
    wk = np.zeros((128, WK_COLS), np.float32)
    for ti, t in enumerate(tiles):
        for a in range(3):
            wk[:, (ti * 3 + a) * 128:(ti * 3 + a + 1) * 128] = t[a]
    wk[:, 1920] = np.tile(bias_np, 2)
    wk[:, 1921] = np.concatenate([np.zeros(64, np.float32), bias_np])  # pair k=-1
    wk[:, 1922] = np.concatenate([bias_np, np.zeros(64, np.float32)])  # pair k=15
    return wk


def _emit_one(nc, zpool, psum, psumb, bsb, obuf, wk_t, mmdt, w_in, b_in, o_w, o_b):
    """Emit the conv + bias pipeline for one [65536, 32] tensor slice."""
    w_r = w_in.rearrange("(y t) n -> t y n", t=2 * W * CO // 64 * 32)  # [2048, 32, 32]
    w_r = w_r.bitcast(mmdt)
    o_r = o_w.rearrange("(y xg xo i) n -> xg xo i y n", y=H, xg=8, xo=4, i=CI)

    z = {}

    def load(kk):  # kk = odd-pair index + 1, covers input x = 2*kk-1, 2*kk
        t = zpool.tile([128, H, NOUT], mmdt, tag="z", name="z")
        if kk == 0:
            # x=-1 half is never weighted (zeroed lhsT/bias cols) but must be
            # finite: fill it with a copy of the valid x=0 rows.
            nc.sync.dma_start(t[0:64], w_r[0:64])
            nc.sync.dma_start(t[64:128], w_r[0:64])
        elif kk == 16:
            nc.sync.dma_start(t[0:64], w_r[1984:2048])
            nc.sync.dma_start(t[64:128], w_r[1984:2048])
        else:
            nc.sync.dma_start(t[:], w_r[64 + (kk - 1) * 128: 64 + kk * 128])
        z[kk] = t

    pbias = [psumb.tile([1, 512], F32, tag="pb", name="pb") for _ in range(2)]

    nextload = 0
    for g in range(8):
        while nextload <= min(2 * g + 2, 16):
            load(nextload)
            nextload += 1
        for h in range(2):
            pt = psum.tile([128, 16, NOUT], F32, tag="ps", name="ps")
            mms = []
            for s in range(3):
                ti = s
                if g == 0 and s == 0:
                    ti = 3
                if g == 7 and s == 2:
                    ti = 4
                for a in range(3):
                    y0 = 16 * h + a - 1
                    yi0, yi1 = max(y0, 0), min(y0 + 16, H)
                    mms.append((ti, a, 2 * g + s, yi0, yi1 - yi0, yi0 - y0))
            mms.sort(key=lambda m: m[1] != 1)  # a full-N matmul first (start=True)
            for i, (ti, a, kk, yi0, cnt, yo) in enumerate(mms):
                nc.tensor.matmul(
                    pt[:, yo:yo + cnt, :],
                    lhsT=wk_t[:, (ti * 3 + a) * 128:(ti * 3 + a + 1) * 128],
                    rhs=z[kk][:, yi0:yi0 + cnt, :],
                    start=(i == 0), stop=(i == len(mms) - 1))
            ot = obuf.tile([128, 16, NOUT], F32, tag="ot", name="ot")
            nc.scalar.copy(ot[:], pt[:])
            nc.sync.dma_start(o_r[g, :, :, 16 * h:16 * h + 16, :], ot[:])
        for kk in [2 * g, 2 * g + 1] + ([16] if g == 7 else []):
            col = 1921 if kk == 0 else (1922 if kk == 16 else 1920)
            for h2 in range(2):
                nc.tensor.matmul(
                    pbias[h2][:, :],
                    lhsT=wk_t[:, col:col + 1],
                    rhs=z[kk][:, 16 * h2:16 * h2 + 16, :],
                    start=(kk == 0), stop=(kk == 16))

    bt = bsb.tile([1, 512], F32, tag="bt", name="bt")
    nc.scalar.copy(bt[:, :], pbias[0][:, :])
    nc.vector.tensor_add(bt[:, :], bt[:, :], pbias[1][:, :])  # one PSUM operand max
    for sz in (256, 128, 64, 32):
        nc.vector.tensor_add(bt[:, 0:sz], bt[:, 0:sz], bt[:, sz:2 * sz])
    binp = bsb.tile([1, NOUT], F32, tag="bin", name="binp")
    nc.sync.dma_start(binp[:, :], b_in[:, :])
    nc.vector.tensor_add(bt[:, 0:NOUT], bt[:, 0:NOUT], binp[:, :])
    nc.sync.dma_start(o_b[:, :], bt[:, 0:NOUT])


def _build_program(mmdt=mybir.dt.float32r, repeat=1):
    nc = bacc.Bacc(trn_type="TRN2", target_bir_lowering=False, debug=False)
    w_u = nc.dram_tensor("w_u", [NF_OUT, NOUT], F32, kind="ExternalInput").ap()
    w_l = nc.dram_tensor("w_l", [NF_OUT, NOUT], F32, kind="ExternalInput").ap()
    b_u = nc.dram_tensor("b_u", [1, NOUT], F32, kind="ExternalInput").ap()
    b_l = nc.dram_tensor("b_l", [1, NOUT], F32, kind="ExternalInput").ap()
    wk = nc.dram_tensor("wk", [128, WK_COLS], F32, kind="ExternalInput").ap()
    o_wu = nc.dram_tensor("o_wu", [NF_IN, NOUT], F32, kind="ExternalOutput").ap()
    o_wl = nc.dram_tensor("o_wl", [NF_IN, NOUT], F32, kind="ExternalOutput").ap()
    o_bu = nc.dram_tensor("o_bu", [1, NOUT], F32, kind="ExternalOutput").ap()
    o_bl = nc.dram_tensor("o_bl", [1, NOUT], F32, kind="ExternalOutput").ap()

    with tile.TileContext(nc) as tc, ExitStack() as ctx:
        wkp = ctx.enter_context(tc.tile_pool(name="wkp", bufs=1))
        zpool = ctx.enter_context(tc.tile_pool(name="z", bufs=8))
        psum = ctx.enter_context(tc.tile_pool(name="ps", bufs=4, space="PSUM"))
        psumb = ctx.enter_context(tc.tile_pool(name="pb", bufs=2, space="PSUM"))
        bsb = ctx.enter_context(tc.tile_pool(name="bsb", bufs=2))
        obuf = ctx.enter_context(tc.tile_pool(name="ob", bufs=3))
        wk_t = wkp.tile([128, WK_COLS], mmdt, name="wk_t")
        nc.sync.dma_start(wk_t[:], wk.bitcast(mmdt)[:, :])
        for _ in range(repeat):
            _emit_one(nc, zpool, psum, psumb, bsb, obuf, wk_t, mmdt, w_u, b_u, o_wu, o_bu)
            _emit_one(nc, zpool, psum, psumb, bsb, obuf, wk_t, mmdt, w_l, b_l, o_wl, o_bl)
    nc.compile()
    return nc


_CACHE: dict = {}


def _get_program():
    if "nc" not in _CACHE:
        _CACHE["nc"] = _build_program()
    return _CACHE["nc"]


def kernel(w_out_u, b_out_u, w_out_l, b_out_l, kernel, bias, _run_kwargs=None):
    w_out_u = np.ascontiguousarray(np.asarray(w_out_u, np.float32))
    w_out_l = np.ascontiguousarray(np.asarray(w_out_l, np.float32))
    b_out_u = np.ascontiguousarray(np.asarray(b_out_u, np.float32))
    b_out_l = np.ascontiguousarray(np.asarray(b_out_l, np.float32))
    wk = _host_weights(np.asarray(kernel, np.float32), np.asarray(bias, np.float32))

    nc = _get_program()
    in_maps = [
        {
            "w_u": w_out_u[b], "w_l": w_out_l[b],
            "b_u": b_out_u[b:b + 1], "b_l": b_out_l[b:b + 1],
            "wk": wk,
        }
        for b in range(B)
    ]
    res = run_bass_kernel_spmd(nc, in_maps, core_ids=list(range(B)),
                               **(_run_kwargs or {}))
    if _run_kwargs:
        _CACHE["last_results"] = res
    w_u = np.stack([res.results[b]["o_wu"] for b in range(B)])
    b_u = np.stack([res.results[b]["o_bu"][0] for b in range(B)])
    w_l = np.stack([res.results[b]["o_wl"] for b in range(B)])
    b_l = np.stack([res.results[b]["o_bl"][0] for b in range(B)])
    return (w_u, b_u, w_l, b_l)


# revision 24
# speedup vs baseline: 113.7098x; 113.7098x over previous
"""Trainium2 Bass kernel for nn_BackwardConv2D (conv_transpose backward pass).

Math (verified vs jax): for each (batch, n_out) slice, with z = w reshaped to
[H, W, C_OUT] (channels-last, flat index (y*W + x)*C_OUT + co):

    out[y, x, ci] = sum_{a,b in 0..2} z_pad[y+a-1, x+b-1, co] * kf[a, b, ci, co]
    kf = kernel[::-1, ::-1]            (spatial flip; zero 'SAME' padding)
    b_new[n]     = sum_{y,x,co} z[y,x,co] * bias[co] + b[n]

Sharding: data-parallel over batch B=8 -> one batch slice per NeuronCore
(each core handles both the _u and _l tensors of its batch).

Per-core implementation: the co=64 contraction is packed two x-columns at a
time into the PE array's K=128 partitions ("odd-pair" tiling: pair k holds
input columns x=2k+1, 2k+2, so the three K-spans feeding a 4-wide output
x-group are all pair-aligned).  Each PSUM fill covers outputs
[128 part = (x%4, ci), 512 free = (16 y, 32 n)] and accumulates 9 matmuls
(3 K-spans x 3 dy-taps; dy shifts are free-dim offsets).  Matmuls run as
float32r (1 cycle/row vs 4 for float32).  The bias reduction reuses the same
SBUF tiles with an M=1 matmul against a replicated bias column.  x-edge taps
are handled by zeroed lhsT variants, y-edge taps by trimming the free range.
All lhsT weight matrices are built host-side from the small kernel/bias
inputs and shipped as one [128, 1923] DRAM tensor.
"""

from contextlib import ExitStack

import numpy as np

import concourse.bass as bass
import concourse.mybir as mybir
import concourse.tile as tile
from concourse import bacc
from concourse.bass_utils import run_bass_kernel_spmd

H = W = 32
CI, CO = 32, 64
B, NOUT = 8, 32
NF_OUT = H * W * CO   # 65536
NF_IN = H * W * CI    # 32768
F32 = mybir.dt.float32
WK_COLS = 15 * 128 + 3  # 15 lhsT tiles + 3 bias columns


def _host_weights(kernel_np: np.ndarray, bias_np: np.ndarray) -> np.ndarray:
    """Build the [128, 1923] stationary-weight matrix from kernel/bias.

    lhsT tile for K-span s (s in 0..2), dy-tap a: [K=128, M=128] where
    K rows = (half, co) for input columns x = 4g-1+2s+half and
    M cols = (j, ci) for output column x = 4g+j.  The x-tap index is
    bt = 2s + half - j (valid 0..2).  Tiles 3/4 are edge variants of
    s=0/s=2 with the out-of-range input half zeroed (x=-1 / x=32).
    """
    kf = kernel_np[::-1, ::-1]  # [a, bt, ci, co]

    def span_lhs(s, zero_half=None):
        out = np.zeros((3, 128, 128), np.float32)
        for a in range(3):
            for half in range(2):
                if half == zero_half:
                    continue
                for j in range(4):
                    bt = 2 * s + half - j
                    if 0 <= bt <= 2:
                        out[a, half * 64:(half + 1) * 64, j * 32:(j + 1) * 32] = kf[a, bt].T
        return out

    tiles = [span_lhs(0), span_lhs(1), span_lhs(2),
             span_lhs(0, zero_half=0), span_lhs(2, zero_half=1)]
    wk = np.zeros((128, WK_COLS), np.float32)
    for ti, t in enumerate(tiles):
        for a in range(3):
            wk[:, (ti * 3 + a) * 128:(ti * 3 + a + 1) * 128] = t[a]
    wk[:, 1920] = np.tile(bias_np, 2)
    wk[:, 1921] = np.concatenate([np.zeros(64, np.float32), bias_np])  # pair k=-1
    wk[:, 1922] = np.concatenate([bias_np, np.zeros(64, np.float32)])  # pair k=15
    return wk


def _emit_one(nc, zpool, psum, psumb, bsb, obuf, wk_t, mmdt, w_in, b_in, o_w, o_b,
              do_conv=True, do_out=True, dma_eng=None):
    if dma_eng is None:
        dma_eng = nc.sync
    """Emit the conv + bias pipeline for one [65536, 32] tensor slice."""
    w_r = w_in.rearrange("(y t) n -> t y n", t=2 * W * CO // 64 * 32)  # [2048, 32, 32]
    w_r = w_r.bitcast(mmdt)
    o_r = o_w.rearrange("(y xg xo i) n -> xg xo i y n", y=H, xg=8, xo=4, i=CI)

    z = {}

    def load(kk):  # kk = odd-pair index + 1, covers input x = 2*kk-1, 2*kk
        t = zpool.tile([128, H, NOUT], mmdt, tag="z", name="z")
        if kk == 0:
            # x=-1 half is never weighted (zeroed lhsT/bias cols) but must be
            # finite: fill it with a copy of the valid x=0 rows.
            dma_eng.dma_start(t[0:64], w_r[0:64])
            dma_eng.dma_start(t[64:128], w_r[0:64])
        elif kk == 16:
            dma_eng.dma_start(t[0:64], w_r[1984:2048])
            dma_eng.dma_start(t[64:128], w_r[1984:2048])
        else:
            dma_eng.dma_start(t[:], w_r[64 + (kk - 1) * 128: 64 + kk * 128])
        z[kk] = t

    pbias = [psumb.tile([1, 512], F32, tag="pb", name="pb") for _ in range(2)]

    nextload = 0
    for g in range(8):
        while nextload <= min(2 * g + 2, 16):
            load(nextload)
            nextload += 1
        if not do_conv:
            continue
        for h in range(2):
            pt = psum.tile([128, 16, NOUT], F32, tag="ps", name="ps")
            mms = []
            for s in range(3):
                ti = s
                if g == 0 and s == 0:
                    ti = 3
                if g == 7 and s == 2:
                    ti = 4
                for a in range(3):
                    y0 = 16 * h + a - 1
                    yi0, yi1 = max(y0, 0), min(y0 + 16, H)
                    mms.append((ti, a, 2 * g + s, yi0, yi1 - yi0, yi0 - y0))
            mms.sort(key=lambda m: m[1] != 1)  # a full-N matmul first (start=True)
            for i, (ti, a, kk, yi0, cnt, yo) in enumerate(mms):
                nc.tensor.matmul(
                    pt[:, yo:yo + cnt, :],
                    lhsT=wk_t[:, (ti * 3 + a) * 128:(ti * 3 + a + 1) * 128],
                    rhs=z[kk][:, yi0:yi0 + cnt, :],
                    start=(i == 0), stop=(i == len(mms) - 1))
            ot = obuf.tile([128, 16, NOUT], F32, tag="ot", name="ot")
            nc.scalar.copy(ot[:], pt[:])
            if do_out:
                dma_eng.dma_start(o_r[g, :, :, 16 * h:16 * h + 16, :], ot[:])
        for kk in [2 * g, 2 * g + 1] + ([16] if g == 7 else []):
            col = 1921 if kk == 0 else (1922 if kk == 16 else 1920)
            for h2 in range(2):
                nc.tensor.matmul(
                    pbias[h2][:, :],
                    lhsT=wk_t[:, col:col + 1],
                    rhs=z[kk][:, 16 * h2:16 * h2 + 16, :],
                    start=(kk == 0), stop=(kk == 16))

    if not do_conv:
        # Diagnostic mode: still produce the b output so DRAM outs are written.
        bt0 = bsb.tile([1, NOUT], F32, tag="bt0", name="bt0")
        nc.sync.dma_start(bt0[:, :], b_in[:, :])
        nc.sync.dma_start(o_b[:, :], bt0[:, :])
        return
    bt = bsb.tile([1, 512], F32, tag="bt", name="bt")
    nc.scalar.copy(bt[:, :], pbias[0][:, :])
    nc.vector.tensor_add(bt[:, :], bt[:, :], pbias[1][:, :])  # one PSUM operand max
    for sz in (256, 128, 64, 32):
        nc.vector.tensor_add(bt[:, 0:sz], bt[:, 0:sz], bt[:, sz:2 * sz])
    binp = bsb.tile([1, NOUT], F32, tag="bin", name="binp")
    nc.sync.dma_start(binp[:, :], b_in[:, :])
    nc.vector.tensor_add(bt[:, 0:NOUT], bt[:, 0:NOUT], binp[:, :])
    nc.sync.dma_start(o_b[:, :], bt[:, 0:NOUT])


def _build_program(mmdt=mybir.dt.float32r, repeat=1, do_conv=True, do_out=True, dma='sync'):
    nc = bacc.Bacc(trn_type="TRN2", target_bir_lowering=False, debug=False)
    w_u = nc.dram_tensor("w_u", [NF_OUT, NOUT], F32, kind="ExternalInput").ap()
    w_l = nc.dram_tensor("w_l", [NF_OUT, NOUT], F32, kind="ExternalInput").ap()
    b_u = nc.dram_tensor("b_u", [1, NOUT], F32, kind="ExternalInput").ap()
    b_l = nc.dram_tensor("b_l", [1, NOUT], F32, kind="ExternalInput").ap()
    wk = nc.dram_tensor("wk", [128, WK_COLS], F32, kind="ExternalInput").ap()
    o_wu = nc.dram_tensor("o_wu", [NF_IN, NOUT], F32, kind="ExternalOutput").ap()
    o_wl = nc.dram_tensor("o_wl", [NF_IN, NOUT], F32, kind="ExternalOutput").ap()
    o_bu = nc.dram_tensor("o_bu", [1, NOUT], F32, kind="ExternalOutput").ap()
    o_bl = nc.dram_tensor("o_bl", [1, NOUT], F32, kind="ExternalOutput").ap()

    with tile.TileContext(nc) as tc, ExitStack() as ctx:
        wkp = ctx.enter_context(tc.tile_pool(name="wkp", bufs=1))
        zpool = ctx.enter_context(tc.tile_pool(name="z", bufs=8))
        psum = ctx.enter_context(tc.tile_pool(name="ps", bufs=4, space="PSUM"))
        psumb = ctx.enter_context(tc.tile_pool(name="pb", bufs=2, space="PSUM"))
        bsb = ctx.enter_context(tc.tile_pool(name="bsb", bufs=2))
        obuf = ctx.enter_context(tc.tile_pool(name="ob", bufs=3))
        wk_t = wkp.tile([128, WK_COLS], mmdt, name="wk_t")
        nc.sync.dma_start(wk_t[:], wk.bitcast(mmdt)[:, :])
        for _ in range(repeat):
            eng = getattr(nc, dma)
            _emit_one(nc, zpool, psum, psumb, bsb, obuf, wk_t, mmdt, w_u, b_u, o_wu, o_bu,
                      do_conv=do_conv, do_out=do_out, dma_eng=eng)
            _emit_one(nc, zpool, psum, psumb, bsb, obuf, wk_t, mmdt, w_l, b_l, o_wl, o_bl,
                      do_conv=do_conv, do_out=do_out, dma_eng=eng)
    nc.compile()
    return nc


_CACHE: dict = {}


def _get_program():
    if "nc" not in _CACHE:
        _CACHE["nc"] = _build_program()
    return _CACHE["nc"]


def kernel(w_out_u, b_out_u, w_out_l, b_out_l, kernel, bias, _run_kwargs=None):
    w_out_u = np.ascontiguousarray(np.asarray(w_out_u, np.float32))
    w_out_l = np.ascontiguousarray(np.asarray(w_out_l, np.float32))
    b_out_u = np.ascontiguousarray(np.asarray(b_out_u, np.float32))
    b_out_l = np.ascontiguousarray(np.asarray(b_out_l, np.float32))
    wk = _host_weights(np.asarray(kernel, np.float32), np.asarray(bias, np.float32))

    nc = _get_program()
    in_maps = [
        {
            "w_u": w_out_u[b], "w_l": w_out_l[b],
            "b_u": b_out_u[b:b + 1], "b_l": b_out_l[b:b + 1],
            "wk": wk,
        }
        for b in range(B)
    ]
    res = run_bass_kernel_spmd(nc, in_maps, core_ids=list(range(B)),
                               **(_run_kwargs or {}))
    if _run_kwargs:
        _CACHE["last_results"] = res
    w_u = np.stack([res.results[b]["o_wu"] for b in range(B)])
    b_u = np.stack([res.results[b]["o_bu"][0] for b in range(B)])
    w_l = np.stack([res.results[b]["o_wl"] for b in range(B)])
    b_l = np.stack([res.results[b]["o_bl"][0] for b in range(B)])
    return (w_u, b_u, w_l, b_l)
